# revision 10
# baseline (speedup 1.0000x reference)
"""Trainium2 Bass kernel for nn_LstmCellS (matrix-state LSTM cell).

Math (per gate g in [f, i, o, c]):
    pre[g] = hidden_u @ Ww[g]^T - x @ Wd[g]^T + hidden_s @ Wu[g]^T + (Bw+Bd+Bu)[g]
    f, i, o = sigmoid(pre[0..2]);  gg = tanh(pre[3])
    c     = f * hidden_c + i * gg
    out_s = o * tanh(c)

Sharding: tensor-parallel over the output axis p (flattened (a, b), S^2 = 4096
-> 512 per core).  Every core sees the full batch and full contraction but only
its 512-wide output slice of every gate, so the whole LSTM epilogue is local —
no collectives.  Host concatenates the 8 output slices.

Per-core matmul: out[n_tile(128), p(512)] accumulated over 40 contraction tiles
of 128, where the contraction axis is the concatenation [hidden_s (4096),
hidden_u (512), x (512)] = 5120 and the weight rows are [Wu, Ww, -Wd].
Stationary operand = transposed activations in bf16 (shared by all 4 gates);
moving operand = transposed weights stored as fp8 e4m3 scaled by 512.  An
e4m3 MOVING operand streams at the full bf16 rate (216 ns per 128x128x512
matmul; e3m4 moving measures ~20% slower) while halving the weight DMA to
~10.5 MiB/core — critical because with 8 cores running, per-core HBM is only
~260 GB/s, so bf16 weights (21 MiB) would starve the PE.  The x512 is divided
back out by the activation's scale input; the bias (pre-scaled x512, bf16) is
folded in via K=1 ones^T @ bias matmuls that run inside the warm stream.
Measured rel-err ~1.4e-2 vs the fp32 reference (gate is 2e-2; the error is
set by the host-side e4m3 weight quantization and is deterministic).

Schedule: the last weight slab is emitted gate-major (g, f, i, o) so the
tanh/sigmoid + cell-update epilogue of the first three gates overlaps the
tail of the matmul stream; only sigmoid(o) -> out_s mul -> OS DMA trail the
last matmul.  Early DMA triggers are spread across the sync/scalar/gpsimd
engines so the first weight pieces and activations land in parallel, and
output DMAs go through the sync engine's hardware DGE queue.
"""

import sys

for _p in ("/root/.axon_site/_ro/trn_rl_repo", "/opt/trn_rl_repo"):
    if _p not in sys.path:
        sys.path.append(_p)

import ml_dtypes
import numpy as np

B = 256          # batch
S2 = 4096        # S*S (flattened matrix state)
U = 512          # hidden_u size
I = 512          # input size
QC = S2 + U + I  # contraction length (5120)
QT = QC // 128   # contraction tiles (40)
NT = B // 128    # batch tiles (2)
NCORES = 8
PSH = S2 // NCORES   # output slice per core (512)
SLABQ = 4            # contraction tiles per weight slab
NSLAB = QT // SLABQ  # weight slabs (10)
WBUFS = 3            # weight slab slots in SBUF
WSCALE = 512.0       # fp8 weight pre-scale (divided out in the activation)
NWARM = 6            # dummy warm-up matmuls (HAM clock-gate)

_cache: dict = {}


def _build():
    """Build and compile the per-core Bass module (same NEFF on all cores)."""
    import concourse.tile as tile
    import concourse.mybir as mybir
    from concourse import bacc

    f32 = mybir.dt.float32
    bf16 = mybir.dt.bfloat16
    f8 = mybir.dt.float8e4
    AF = mybir.ActivationFunctionType

    nc = bacc.Bacc("TRN2", target_bir_lowering=False, debug=False,
                   enable_asserts=False, num_devices=NCORES)

    A_d = nc.dram_tensor("A", [128, QT * B], bf16, kind="ExternalInput")
    W_d = nc.dram_tensor("W", [NSLAB, 128, SLABQ * 4 * PSH], f8,
                         kind="ExternalInput")
    B_d = nc.dram_tensor("BIAS", [1, 4 * PSH], bf16, kind="ExternalInput")
    H_d = nc.dram_tensor("HC", [128, NT * PSH], f32, kind="ExternalInput")
    OS_d = nc.dram_tensor("OS", [128, NT * PSH], f32, kind="ExternalOutput")
    CO_d = nc.dram_tensor("CO", [128, NT * PSH], f32, kind="ExternalOutput")

    with tile.TileContext(nc) as tc:
        with (
            tc.tile_pool(name="apool", bufs=1) as apool,
            tc.tile_pool(name="wpool", bufs=WBUFS) as wpool,
            tc.tile_pool(name="cpool", bufs=1) as cpool,
            tc.tile_pool(name="epool", bufs=2) as epool,
            tc.tile_pool(name="pspool", bufs=1, space="PSUM") as pspool,
        ):
            # PSUM accumulators: one bank per (gate, batch-tile)
            psum = [
                pspool.tile([128, PSH], f32, tag=f"ps{g}_{n}", name=f"ps{g}_{n}")
                for g in range(4) for n in range(NT)
            ]

            # Early DMAs, spread across engines so the triggers issue in
            # parallel: sync takes weight pieces 0/2 (then the big slabs),
            # scalar takes the activations + pieces 1/3, gpsimd takes
            # bias + hidden_c.
            ones_t = cpool.tile([1, 128], bf16, tag="ones")
            nc.gpsimd.memset(ones_t[:], 1.0)
            bias_t = cpool.tile([1, 4 * PSH], bf16, tag="bias")
            nc.gpsimd.dma_start(bias_t[:], B_d.ap()[:])
            hc_t = cpool.tile([128, NT * PSH], f32, tag="hc")

            # Slab 0 as 4 separate one-q-tile pieces (256 KiB each) so the
            # very first matmuls wait on as little DMA as possible; triggers
            # are spread across sync and scalar so they issue in parallel.
            # Nothing else competes for HBM in this window: the A chunks and
            # hidden_c are deliberately deferred (weight slabs pace the PE).
            P4 = 4 * PSH
            w0 = [wpool.tile([128, P4], f8, tag=f"w0{h}", name=f"w0{h}")
                  for h in range(SLABQ)]
            nc.sync.dma_start(w0[0][:], W_d.ap()[0][:, 0:P4])
            a0a = apool.tile([128, 2 * B], bf16, tag="a0a", name="a0a")
            nc.scalar.dma_start(a0a[:], A_d.ap()[:, 0:2 * B])
            nc.scalar.dma_start(w0[1][:], W_d.ap()[0][:, P4:2 * P4])
            nc.sync.dma_start(w0[2][:], W_d.ap()[0][:, 2 * P4:3 * P4])
            a0b = apool.tile([128, 8 * B], bf16, tag="a", bufs=3, name="a0b")
            nc.scalar.dma_start(a0b[:], A_d.ap()[:, 2 * B:10 * B])
            nc.scalar.dma_start(w0[3][:], W_d.ap()[0][:, 3 * P4:4 * P4])

            # HAM warm-up: tiny K=1 dummy matmuls (ones x ones) into psum[0]
            # (reset by the start=True matmuls of q-tile 0) so the PE
            # activity window opens while the first weight piece is still
            # in flight.
            for _ in range(NWARM):
                nc.tensor.matmul(
                    psum[0][:, :128], ones_t[:], ones_t[:],
                    start=True, stop=True, skip_group_check=True)

            # A tiles: a0a covers q-tiles 0-1, a0b 2-9, then 10-per-chunk.
            a_tiles = [None] * 3

            def lhs_for(qt, n):
                if qt < 2:
                    return a0a[:, qt * B + n * 128: qt * B + (n + 1) * 128]
                if qt < 10:
                    off = qt - 2
                    return a0b[:, off * B + n * 128: off * B + (n + 1) * 128]
                ch = (qt - 10) // 10
                off = qt - 10 - ch * 10
                return a_tiles[ch][:, off * B + n * 128: off * B + (n + 1) * 128]

            def emit_mms(qt, wtile, base, gates=range(4)):
                for n in range(NT):
                    lhs = lhs_for(qt, n)
                    for g in gates:
                        nc.tensor.matmul(
                            psum[g * NT + n][:],
                            lhs,
                            wtile[:, base + g * PSH: base + (g + 1) * PSH],
                            start=(qt == 0),
                            stop=(qt == QT - 1),
                        )

            # Slab 0 (from the 4 pieces)
            for h in range(SLABQ):
                emit_mms(h, w0[h], 0)

            # Bias via K=1 matmul: psum[g,n] += ones[1,128]^T @ bias[1,512].
            # Emitted after slab 0 so these 8 matmuls run inside the warm
            # stream instead of cold-blocking it at the head.
            for n in range(NT):
                for g in range(4):
                    nc.tensor.matmul(
                        psum[g * NT + n][:], ones_t[:],
                        bias_t[:, g * PSH:(g + 1) * PSH],
                        start=False, stop=False)

            # Epilogue helpers -------------------------------------------------
            acts = [[None] * 4 for _ in range(NT)]   # [n][g]
            fhc = [None] * NT
            ig = [None] * NT
            th = [None] * NT
            c_t = epool.tile([128, NT * PSH], f32, tag="ct", bufs=1, name="ct")
            os_t = epool.tile([128, NT * PSH], f32, tag="ost", bufs=1, name="ost")

            def emit_act(g, n):
                a = epool.tile([128, PSH], f32, tag=f"act{g}", name=f"act{g}_{n}")
                nc.scalar.activation(
                    a[:], psum[g * NT + n][:],
                    AF.Tanh if g == 3 else AF.Sigmoid, scale=1.0 / WSCALE)
                acts[n][g] = a

            # Main slab stream.  A chunks and hidden_c are triggered late —
            # each A chunk ~2 slabs before its first consumer — so the weight
            # slabs (which pace the PE) get the DMA bandwidth first.
            TAILJ = NSLAB - 2        # last two slabs are emitted gate-major
            wtail = [None, None]
            for j in range(1, NSLAB):
                if j in (2, 4, 6):
                    ch = j // 2 - 1
                    at = apool.tile([128, 10 * B], bf16, tag="a", bufs=3,
                                    name=f"a{ch}")
                    nc.scalar.dma_start(
                        at[:], A_d.ap()[:, (10 + ch * 10) * B:(20 + ch * 10) * B])
                    a_tiles[ch] = at
                if j == 7:
                    nc.gpsimd.dma_start(hc_t[:], H_d.ap()[:])
                wt = wpool.tile([128, SLABQ * 4 * PSH], f8, tag="w", name=f"w{j}")
                nc.sync.dma_start(wt[:], W_d.ap()[j])
                if j < TAILJ:
                    for h in range(SLABQ):
                        emit_mms(SLABQ * j + h, wt, h * P4)
                    continue
                wtail[j - TAILJ] = wt
                if j < NSLAB - 1:
                    continue

                # Last two slabs: gate-major (g, f, i, o) so the cell update
                # completes while f/i/o matmuls still stream; only sigmoid(o)
                # and the out_s mul trail the last matmul.
                for g in (3, 0, 1, 2):
                    for jj in range(2):
                        for h in range(SLABQ):
                            emit_mms(SLABQ * (TAILJ + jj) + h, wtail[jj],
                                     h * P4, gates=(g,))
                    if g == 3:          # cell candidate done
                        for n in range(NT):
                            emit_act(3, n)
                    elif g == 0:        # f done
                        for n in range(NT):
                            emit_act(0, n)
                            fhc[n] = epool.tile([128, PSH], f32, tag="fhc",
                                                name=f"fhc{n}")
                            nc.vector.tensor_mul(
                                fhc[n][:], acts[n][0][:],
                                hc_t[:, n * PSH:(n + 1) * PSH])
                    elif g == 1:        # i done
                        for n in range(NT):
                            emit_act(1, n)
                            ig[n] = epool.tile([128, PSH], f32, tag="ig",
                                               name=f"ig{n}")
                            nc.vector.tensor_mul(
                                ig[n][:], acts[n][1][:], acts[n][3][:])
                            nc.vector.tensor_add(
                                c_t[:, n * PSH:(n + 1) * PSH],
                                fhc[n][:], ig[n][:])
                            th[n] = epool.tile([128, PSH], f32, tag="th",
                                               name=f"th{n}")
                            nc.scalar.activation(
                                th[n][:], c_t[:, n * PSH:(n + 1) * PSH],
                                AF.Tanh)
                        nc.sync.dma_start(CO_d.ap()[:], c_t[:])

            # o trails the stream: sigmoid -> out_s mul -> OS DMA.
            for n in range(NT):
                emit_act(2, n)
                nc.vector.tensor_mul(
                    os_t[:, n * PSH:(n + 1) * PSH], acts[n][2][:], th[n][:])
            nc.sync.dma_start(OS_d.ap()[:], os_t[:])

    nc.compile()
    return nc


def _get_nc():
    if "nc" not in _cache:
        _cache["nc"] = _build()
    return _cache["nc"]


def _prep_in_maps(x, hidden_s, hidden_u, hidden_c, Wd, Wu, Ww, Bd, Bu, Bw):
    bf16 = ml_dtypes.bfloat16
    f8 = ml_dtypes.float8_e4m3

    # Activations, transposed: A_T[k, n], k = [hs (4096) | hu (512) | x (512)]
    A = np.concatenate(
        [hidden_s.reshape(B, S2), hidden_u, x], axis=1).astype(bf16)     # [B, QC]
    A_sb = np.ascontiguousarray(
        A.T.reshape(QT, 128, B).transpose(1, 0, 2)).reshape(128, QT * B)

    # Weights, transposed to [k, p], scaled by WSCALE into fp8 e4m3 range,
    # gates interleaved in the free dim, SLABQ contraction tiles per slab.
    WuT = Wu.reshape(4, S2, S2).transpose(0, 2, 1)                       # [4,S2,S2]
    WwT = Ww.reshape(4, S2, U).transpose(0, 2, 1)                        # [4,U,S2]
    WdT = (-Wd.reshape(4, S2, I)).transpose(0, 2, 1)                     # [4,I,S2]
    WT = (np.concatenate([WuT, WwT, WdT], axis=1) * WSCALE)              # [4,QC,S2]
    W_r = WT.reshape(4, NSLAB, SLABQ, 128, S2).transpose(1, 3, 2, 0, 4)  # [NS,128,SQ,4,S2]

    bias = ((Bw + Bd + Bu) * WSCALE).reshape(4, S2).astype(np.float32)
    hc = hidden_c.reshape(NT, 128, S2).astype(np.float32)

    in_maps = []
    for c in range(NCORES):
        p0 = c * PSH
        W_c = np.ascontiguousarray(W_r[..., p0:p0 + PSH]).astype(f8).reshape(
            NSLAB, 128, SLABQ * 4 * PSH)
        b_c = np.ascontiguousarray(
            bias[:, p0:p0 + PSH]).reshape(1, 4 * PSH).astype(bf16)
        h_c = np.ascontiguousarray(
            hc[..., p0:p0 + PSH].transpose(1, 0, 2)).reshape(128, NT * PSH)
        in_maps.append({"A": A_sb, "W": W_c, "BIAS": b_c, "HC": h_c})
    return in_maps


def _run(inputs, trace=False, trace_kwargs=None):
    from concourse.bass_utils import run_bass_kernel_spmd

    nc = _get_nc()
    in_maps = _prep_in_maps(**inputs)
    res = run_bass_kernel_spmd(
        nc, in_maps, core_ids=list(range(NCORES)),
        trace=trace, **(trace_kwargs or {}))

    out_s = np.empty((B, S2), np.float32)
    c_out = np.empty((B, S2), np.float32)
    for c in range(NCORES):
        p0 = c * PSH
        out_s[:, p0:p0 + PSH] = (
            res.results[c]["OS"].reshape(128, NT, PSH)
            .transpose(1, 0, 2).reshape(B, PSH))
        c_out[:, p0:p0 + PSH] = (
            res.results[c]["CO"].reshape(128, NT, PSH)
            .transpose(1, 0, 2).reshape(B, PSH))
    return (out_s.reshape(B, 64, 64), c_out.reshape(B, 64, 64)), res


def kernel(**inputs):
    inputs = {k: np.asarray(v) for k, v in inputs.items()}
    (out_s, c_out), _ = _run(inputs)
    return (out_s, c_out)
